# revision 16
# baseline (speedup 1.0000x reference)
"""Trainium2 Bass kernel for AntisymmetricRNN scan.

Reference computation (per batch column b, independent chains):
    A   = triu(W,1) - triu(W,1)^T - 0.001*I          (256x256)
    X_0 = X0^T (n=256, bs=256)
    Y_t = A @ X_t + by
    X_{t+1} = X_t + 0.01*tanh(Y_t),  t = 0..998
    out = stack([X_0 .. X_999]) -> (bs, tmax, n) = (256, 1000, 256)

Strategy (data-parallel over batch, 8 cores, bs=32 per core):
  - The scan is latency-bound: wall time = 999 x (tanh -> matmul -> tanh
    cycle latency).  Sub-chain parallelism cannot reduce it (every chain
    still needs 999 serial steps), so all effort goes into the cycle.
  - Keep Y in PSUM as a running fp32 accumulator:
        Y_{t+1} = Y_t + (0.01*A) @ tanh(Y_t)
    (linearity of A@) so the recurrence is a pure ACT->PE cycle.
  - DTYPE="f16" is the big lever: fp32 nc.tensor.matmul lowers to a
    SELF-LOADING InstMatmult (~440ns each, weight-load+fill+drain
    serialized); fp16 emits separate LDWEIGHTS (4-XBUS fast-weight-load)
    + MATMUL that pipeline at ~10ns each.  Measured 43ns/step for the 4
    MMs vs 1756ns in fp32.  PSUM accumulation stays fp32; the Y0 init
    matmuls run in fp32 for an exact seed.
  - CORR_K=8 cancels the systematic fp16 weight-rounding error: S
    accumulates sum(G)/1024 in fp16 (DVE, off the critical cycle), and
    every 8 steps Y += (1024*(B - fp16(B))) @ S via 4 extra accumulating
    fp16 matmuls (~free).  Device rel err 1.3e-3 (gate 2e-2).
  - BANKALT: the two 128-row halves of Y live in DIFFERENT PSUM banks so
    consecutive matmuls alternate banks and their drains overlap.
  - OUTPUT="g16": per step just 1 ACT tanh (PSUM -> fp16 SBUF ring slab,
    which doubles as the matmuls' G buffer) + 4 matmul-accumulates;
    slabs are DMAed to DRAM every RB steps (overlapped, 3-deep ring).
    The host reconstructs X = X0 + 0.01*cumsum(G) in fp32 - an exact
    reassociation of the reference's X update (diff ~1.6e-6).
  - Host does input prep (skew-parametrization, fp16 split, chunking,
    batch sharding) and output unshard/transpose/cumsum.
  - Measured ~753 ns/step (~0.75ms total) vs 2371 ns/step for the fp32
    baseline kernel (3.1x); remaining cycle = ACT latency (~350ns) +
    2 semaphore hops (~100ns each) + MM fills + PSUM drain (~175ns),
    all serial-latency terms a batch-sharded scan cannot avoid.
"""

import numpy as np

N = 256
BS = 256
TMAX = 1000
STEP = 0.01
EPS = 0.001
NCORES = 8
BSH = BS // NCORES  # 32 batch columns per core
NSTEPS = TMAX - 1   # 999 device steps
H = 2               # n-halves (256 = 2 x 128 partitions)

# Tunables
SUBS = 1            # independent sub-chains per core (must divide BSH)
RB = 37             # ring-batch: steps per output DMA (999 = 27 * 37)
GBUFS = 4           # G tile double-buffering depth
SLAB_BUFS = 3       # output slab buffering depth
VARIANT = "base"    # base | noact | nostt  (diagnostics)
BANKALT = True      # Y halves in different PSUM banks
CT = 0              # column-tile width for per-step MMs (0=off | 64 | 32):
                    # M-tiles go to distinct array col-groups so their
                    # fills/drains overlap (tile_position auto-derived)
DTYPE = "f16"       # f32 | f16 per-step matmul datapath
CORR_K = 8          # f16 only: weight-error correction period (0 = off)
CORR_SC = 1024.0    # correction scale
OUTPUT = "g16"      # g16: device streams tanh values (fp16) straight from
                    # ACT into the DMA ring and the HOST reconstructs
                    # X = X0 + h*cumsum(G) (exact reassociation of the
                    # reference's X update); kills the per-step DVE STT
                    # chain + G-tile recycling from the critical cycle.
                    # x32: legacy on-device X update (fp32 slabs).


def _build_graph(repeat=1):
    import concourse.bass as bass
    import concourse.tile as tile
    from concourse import bacc, mybir

    f32 = mybir.dt.float32
    f16 = mybir.dt.float16
    gdt = f16 if DTYPE == "f16" else f32
    nc = bacc.Bacc("TRN2", target_bir_lowering=False, debug=False,
                   num_devices=NCORES)

    apack_d = nc.dram_tensor("apack", [128, 4 * 128], f32, kind="ExternalInput")
    x0s_d = nc.dram_tensor("x0s", [128, H, BSH], f32, kind="ExternalInput")
    x0_d = nc.dram_tensor("x0", [128, H, BSH], f32, kind="ExternalInput")
    byf_d = nc.dram_tensor("byf", [128, H, BSH], f32, kind="ExternalInput")
    if DTYPE == "f16":
        bh_d = nc.dram_tensor("bhpack", [128, 4 * 128], f16,
                              kind="ExternalInput")
        if CORR_K:
            dc_d = nc.dram_tensor("dcpack", [128, 4 * 128], f16,
                                  kind="ExternalInput")
    if OUTPUT == "g16":
        xout_d = nc.dram_tensor("gout", [128, NSTEPS, H, BSH], f16,
                                kind="ExternalOutput")
    else:
        xout_d = nc.dram_tensor("xout", [128, NSTEPS, H, BSH], f32,
                                kind="ExternalOutput")

    bw = BSH // SUBS  # batch columns per sub-chain

    with tile.TileContext(nc) as tc:
        with tc.tile_pool(name="const", bufs=1) as cpool, \
             tc.tile_pool(name="g", bufs=GBUFS) as gpool, \
             tc.tile_pool(name="slab", bufs=SLAB_BUFS) as spool, \
             tc.tile_pool(name="ypsum", bufs=1, space="PSUM") as ypool:

            a_sb = cpool.tile([128, 4 * 128], f32)
            x0s_sb = cpool.tile([128, H, BSH], f32)
            x0_sb = cpool.tile([128, H, BSH], f32)
            byf_sb = cpool.tile([128, H, BSH], f32)
            nc.sync.dma_start(out=a_sb[:, :], in_=apack_d[:, :])
            nc.sync.dma_start(out=x0s_sb[:, :, :], in_=x0s_d[:, :, :])
            nc.sync.dma_start(out=x0_sb[:, :, :], in_=x0_d[:, :, :])
            nc.sync.dma_start(out=byf_sb[:, :, :], in_=byf_d[:, :, :])
            bh_sb = dc_sb = s_sb = None
            if DTYPE == "f16":
                bh_sb = cpool.tile([128, 4 * 128], f16)
                nc.sync.dma_start(out=bh_sb[:, :], in_=bh_d[:, :])
                if CORR_K:
                    dc_sb = cpool.tile([128, 4 * 128], f16)
                    nc.sync.dma_start(out=dc_sb[:, :], in_=dc_d[:, :])
                    s_sb = cpool.tile([128, H, BSH], f16)
                    nc.vector.memset(s_sb[:, :, :], 0.0)

            # stationary chunk (k, m) of lhsT layout [K=n_in, M=n_out]
            def chunk(sb, k, m):
                c = 2 * k + m
                return sb[:, 128 * c:128 * (c + 1)]

            ach = lambda k, m: chunk(a_sb, k, m)          # noqa: E731
            bhch = (lambda k, m: chunk(bh_sb, k, m)) if bh_sb is not None \
                else ach
            dcch = (lambda k, m: chunk(dc_sb, k, m)) if dc_sb is not None \
                else None

            # Per-sub-chain Y accumulator.  BANKALT lays each [128, H, bw]
            # accumulator out as [128, H, 512] so the two n-halves land in
            # DIFFERENT PSUM banks - consecutive matmuls then alternate
            # banks and their drains pipeline.
            if BANKALT:
                _yt = [ypool.tile([128, H, 512], f32, name=f"y{j}")
                       for j in range(SUBS)]
                ys = [yt[:, :, 0:bw] for yt in _yt]
            else:
                ys = [ypool.tile([128, H, bw], f32, name=f"y{j}")
                      for j in range(SUBS)]

            # Y_0 = (0.01*A) @ (100*X0) + by, in fp32.  First MM per PSUM
            # bank is start=True; everything else accumulates.
            for j in range(SUBS):
                bsl = slice(j * bw, (j + 1) * bw)
                for k in range(H):
                    for m in range(H):
                        nc.tensor.matmul(
                            ys[j][:, m, :], ach(k, m), x0s_sb[:, k, bsl],
                            start=(k == 0) if BANKALT
                            else (m == 0 and k == 0),
                            stop=False, skip_group_check=True)
                nc.vector.tensor_add(ys[j][:, :, :], ys[j][:, :, :],
                                     byf_sb[:, :, bsl])

            xprev = [x0_sb[:, :, slice(j * bw, (j + 1) * bw)]
                     for j in range(SUBS)]

            body_args = (nc, tc, mybir, gpool, spool, ys, xprev, xout_d,
                         bhch, dcch, s_sb, gdt)
            if repeat > 1:
                # dynamic loop for wall-clock benchmarking only: same
                # instruction count as repeat=1, ~2-6us back-edge.
                with tc.For_i(0, repeat, 1):
                    _loop_body(*body_args)
            else:
                _loop_body(*body_args)

    nc.compile()
    return nc


def _loop_body(nc, tc, mybir, gpool, spool, ys, xprev, xout_d,
               bhch, dcch, s_sb, gdt):
    bw = BSH // SUBS
    f32 = mybir.dt.float32
    g16mode = (OUTPUT == "g16") and VARIANT == "base"
    gconst = None
    t = 0
    while t < NSTEPS:
        nb = min(RB, NSTEPS - t)
        slab = spool.tile([128, RB, H, BSH], gdt if g16mode else f32)
        for s in range(nb):
            st = t + s
            last = (st == NSTEPS - 1)
            for j in range(SUBS):
                bsl = slice(j * bw, (j + 1) * bw)

                # periodic fp16 weight-error correction (before this
                # step's tanh; uses S = sum(G)/SC over prior steps)
                if dcch is not None and st > 0 and st % CORR_K == 0:
                    for k in range(H):
                        for m in range(H):
                            nc.tensor.matmul(
                                ys[j][:, m, :], dcch(k, m),
                                s_sb[:, k, bsl],
                                start=False, stop=False,
                                skip_group_check=True)
                    nc.vector.memset(s_sb[:, :, bsl], 0.0)

                if g16mode:
                    # tanh goes straight into the fp16 output ring slab;
                    # the slab doubles as the G buffer for the matmuls.
                    g = slab[:, s, :, bsl]
                    nc.scalar.activation(
                        g[:, :, :], ys[j][:, :, :],
                        mybir.ActivationFunctionType.Tanh)
                elif "noact" in VARIANT:
                    g = gpool.tile([128, H, bw], gdt, tag=f"g{j}")
                    if st == 0:
                        nc.scalar.activation(
                            g[:, :, :], ys[j][:, :, :],
                            mybir.ActivationFunctionType.Tanh)
                        gconst = g
                    g = gconst
                else:
                    g = gpool.tile([128, H, bw], gdt, tag=f"g{j}")
                    nc.scalar.activation(
                        g[:, :, :], ys[j][:, :, :],
                        mybir.ActivationFunctionType.Tanh)
                if "nomm" in VARIANT:
                    pass  # diagnostic: ACT+STT only, no recurrence MMs
                elif not last:
                    # k-outer, m-inner: consecutive MMs write alternating
                    # Y halves (different banks under BANKALT)
                    for k in range(H):
                        for m in range(H):
                            if CT:
                                for j2 in range(128 // CT):
                                    psl = slice(CT * j2, CT * (j2 + 1))
                                    nc.tensor.matmul(
                                        ys[j][psl, m, :],
                                        bhch(k, m)[:, psl], g[:, k, :],
                                        start=False, stop=False,
                                        skip_group_check=True)
                            else:
                                nc.tensor.matmul(
                                    ys[j][:, m, :], bhch(k, m), g[:, k, :],
                                    start=False, stop=False,
                                    skip_group_check=True)
                    if s_sb is not None:
                        nc.vector.scalar_tensor_tensor(
                            out=s_sb[:, :, bsl], in0=g[:, :, :],
                            scalar=1.0 / CORR_SC, in1=s_sb[:, :, bsl],
                            op0=mybir.AluOpType.mult,
                            op1=mybir.AluOpType.add)
                if (not g16mode and VARIANT != "nostt"
                        and "noact" not in VARIANT):
                    xnew = slab[:, s, :, bsl]
                    nc.vector.scalar_tensor_tensor(
                        out=xnew, in0=g[:, :, :], scalar=STEP,
                        in1=xprev[j], op0=mybir.AluOpType.mult,
                        op1=mybir.AluOpType.add)
                    xprev[j] = xnew
        if VARIANT in ("nostt",) or "noact" in VARIANT:
            if t == 0:
                nc.vector.memset(slab[:, 0, :, :], 0.0)
                nc.sync.dma_start(out=xout_d[:, 0:1, :, :],
                                  in_=slab[:, 0:1, :, :])
        else:
            nc.sync.dma_start(out=xout_d[:, t:t + nb, :, :],
                              in_=slab[:, :nb, :, :])
        t += nb


def _prep_inputs(X0, W, by):
    """Host-side input prep; returns per-core in_maps."""
    X0 = np.asarray(X0, dtype=np.float32)
    W = np.asarray(W, dtype=np.float32)
    by = np.asarray(by, dtype=np.float32).reshape(N, 1)

    U = np.triu(W, 1)
    A = (U - U.T) - np.float32(EPS) * np.eye(N, dtype=np.float32)
    B = (np.float32(STEP) * A).astype(np.float32)

    def pack(M):  # lhsT layout chunks: [K=n_in, M=n_out]
        MT = M.T
        out = np.empty((128, 4 * 128), dtype=M.dtype)
        for k in range(H):
            for m in range(H):
                c = 2 * k + m
                out[:, 128 * c:128 * (c + 1)] = \
                    MT[128 * k:128 * (k + 1), 128 * m:128 * (m + 1)]
        return out

    apack = pack(B)
    extras = {}
    if DTYPE == "f16":
        Bh = B.astype(np.float16)
        extras["bhpack"] = pack(Bh)
        if CORR_K:
            Dc = (np.float32(CORR_SC)
                  * (B - Bh.astype(np.float32))).astype(np.float16)
            extras["dcpack"] = pack(Dc)

    byf = np.empty((128, H, BSH), dtype=np.float32)
    for h in range(H):
        byf[:, h, :] = by[128 * h:128 * (h + 1), 0:1]

    in_maps = []
    for c in range(NCORES):
        Xs = X0[c * BSH:(c + 1) * BSH, :].T.astype(np.float32)  # [n, bsh]
        x0p = np.empty((128, H, BSH), dtype=np.float32)
        for h in range(H):
            x0p[:, h, :] = Xs[128 * h:128 * (h + 1), :]
        in_maps.append({
            "apack": apack,
            "x0s": (np.float32(1.0 / STEP) * x0p).astype(np.float32),
            "x0": x0p,
            "byf": byf,
            **extras,
        })
    return in_maps


_CACHED_NC = None


def _get_nc():
    global _CACHED_NC
    if _CACHED_NC is None:
        _CACHED_NC = _build_graph()
    return _CACHED_NC


def kernel(X0, W, by, _trace=False, _return_results=False):
    from concourse.bass_utils import run_bass_kernel_spmd

    nc = _get_nc()
    in_maps = _prep_inputs(X0, W, by)
    res = run_bass_kernel_spmd(nc, in_maps, core_ids=list(range(NCORES)),
                               trace=_trace)

    out = np.empty((BS, TMAX, N), dtype=np.float32)
    X0 = np.asarray(X0, dtype=np.float32)
    out[:, 0, :] = X0
    for c in range(NCORES):
        if OUTPUT == "g16":
            arr = res.results[c]["gout"]  # [128, 999, H, BSH] fp16
            # n = h*128 + p  ->  (b, t, n)
            G = np.transpose(arr, (3, 1, 2, 0)).reshape(
                BSH, NSTEPS, N).astype(np.float32)
            # X_t = X0 + STEP * sum_{s<t} G_s  (exact reassociation of
            # the reference's sequential X update; G is the same fp16
            # tanh stream the device fed back through the matmuls)
            X = np.cumsum(G, axis=1, dtype=np.float32)
            X *= np.float32(STEP)
            X += X0[c * BSH:(c + 1) * BSH, None, :]
            out[c * BSH:(c + 1) * BSH, 1:, :] = X
        else:
            arr = res.results[c]["xout"]  # [128, 999, H, BSH] = (p, t, h, b)
            arr = np.transpose(arr, (3, 1, 2, 0)).reshape(BSH, NSTEPS, N)
            out[c * BSH:(c + 1) * BSH, 1:, :] = arr
    if _return_results:
        return out, res
    return out
